# revision 15
# baseline (speedup 1.0000x reference)
"""Trainium2 Bass kernel for a causal multi-head attention block.

Computes (per nn.Module reference):
    xn = RMSNorm(x) * g
    q, k, v = split_heads(xn @ Wq), split_heads(xn @ Wkv)
    q, k = rope(q), rope(k)
    out = causal_softmax(q k^T / sqrt(dh)) @ v
    return merge_heads(out) @ Wo

Sharding over 8 NeuronCores: core c handles batch (c // 4) and the
4-head group (c % 4).  Each core computes its head-group's attention
output and a partial out-projection y_c = attn_heads @ Wo[head_slice];
the host sums the 4 partials per batch (the tensor-parallel
all-reduce, done on the host as part of unsharding).

v2 layout: all matmul operands are bf16 (fp32 PSUM accumulation).
x arrives both natural (for RMS stats) and pre-transposed from the
host (no PE transposes).  q/k/v/attn stay SBUF-resident between
phases (no DRAM spills).  Softmax normalization is folded into the
attention-output evacuation, deferred one block to keep PE dense.
"""

import contextlib
import math
import os

os.environ.setdefault("JAX_PLATFORMS", "axon")

import numpy as np

# hardcoded problem shapes (nn_Attention_369367187558)
B = 2          # batch
N = 2048       # sequence length
D = 2048       # model dim
H = 16         # heads
DH = 128       # head dim
HPC = 4        # heads per core
IC = HPC * DH  # inner dim per core (512)
NCORES = 8
GRP = 512      # token group size for phase 1
NGRP = N // GRP
KT = D // 128  # 16 contraction tiles
NT = N // 128  # 16 token tiles
EPS = 1e-8
ATT_SCALE = 1.0 / math.sqrt(DH)

_CACHE = {}


def _build():
    import concourse.mybir as mybir
    import concourse.tile as tile
    from concourse import bacc
    from concourse.masks import make_identity

    F32 = mybir.dt.float32
    F32R = mybir.dt.float32r
    BF16 = mybir.dt.bfloat16
    EXP = mybir.ActivationFunctionType.Exp
    SQRT = mybir.ActivationFunctionType.Sqrt
    SQUARE = mybir.ActivationFunctionType.Square
    COPY = mybir.ActivationFunctionType.Copy

    nc = bacc.Bacc(None, target_bir_lowering=False)

    xT_d = nc.dram_tensor("xT", [D, N], BF16, kind="ExternalInput")
    xn_d = nc.dram_tensor("xn", [N, D], BF16, kind="ExternalInput")
    wq_d = nc.dram_tensor("wq", [D, IC], BF16, kind="ExternalInput")
    wk_d = nc.dram_tensor("wk", [D, IC], BF16, kind="ExternalInput")
    wv_d = nc.dram_tensor("wv", [D, IC], BF16, kind="ExternalInput")
    wo_d = nc.dram_tensor("wo", [IC, D], BF16, kind="ExternalInput")
    cos_d = nc.dram_tensor("cosT", [DH, N], F32, kind="ExternalInput")
    sin_d = nc.dram_tensor("sinTs", [DH, N], F32, kind="ExternalInput")
    mask_d = nc.dram_tensor("mask", [128, 128], BF16, kind="ExternalInput")
    out_d = nc.dram_tensor("out", [N, D], BF16, kind="ExternalOutput")

    NPT = GRP // 128  # token tiles per group

    with tile.TileContext(nc) as tc:
        with (
            tc.tile_pool(name="const", bufs=1) as cpool,
            tc.tile_pool(name="res", bufs=1) as respool,
        ):
            ident = cpool.tile([128, 128], F32, tag="ident")
            make_identity(nc, ident[:])
            ones_col = cpool.tile([128, 1], BF16, tag="onesc")
            nc.vector.memset(ones_col[:], 1.0)
            ones_rf = cpool.tile([1, 128], F32, tag="onesrf")
            nc.vector.memset(ones_rf[:], 1.0)
            ones_row = cpool.tile([1, 128], F32, tag="onesr")
            nc.vector.tensor_copy(ones_row[:].bitcast(F32R),
                                  ones_rf[:].bitcast(F32R))
            mask = cpool.tile([128, 128], BF16, tag="mask")
            nc.sync.dma_start(out=mask[:], in_=mask_d[:, :])

            # SBUF-resident between phases
            qr = respool.tile([128, HPC, N], BF16, tag="qr")
            kr = respool.tile([128, HPC, N], BF16, tag="kr")
            v_res = respool.tile([128, NT, IC], BF16, tag="vres")

            # ------- Phase 1: RMS stats + QKV + rope -------
            with contextlib.ExitStack() as p1st:
                ec = p1st.enter_context
                wpool = ec(tc.tile_pool(name="p1w", bufs=1))
                xtpool = ec(tc.tile_pool(name="p1xt", bufs=2))
                xnpool = ec(tc.tile_pool(name="p1xn", bufs=2))
                sqpool = ec(tc.tile_pool(name="p1sq", bufs=2))
                spool = ec(tc.tile_pool(name="p1s", bufs=2))
                scolpool = ec(tc.tile_pool(name="p1scol", bufs=8))
                srpool = ec(tc.tile_pool(name="p1sr", bufs=2))
                cspool = ec(tc.tile_pool(name="p1cs", bufs=2))
                bbpool = ec(tc.tile_pool(name="p1bb", bufs=3))
                t1pool = ec(tc.tile_pool(name="p1t1", bufs=3))
                rotpool = ec(tc.tile_pool(name="p1rot", bufs=3))
                qkpool = ec(tc.tile_pool(name="p1qk", bufs=3, space="PSUM"))
                vppool = ec(tc.tile_pool(name="p1v", bufs=2, space="PSUM"))
                sbpool = ec(tc.tile_pool(name="p1sb", bufs=1, space="PSUM"))
                stppool = ec(tc.tile_pool(name="p1stp", bufs=2, space="PSUM"))
                # first xT tiles ride the sync queue ahead of the weights,
                # which go out on the gpsimd SWDGE queues
                wq_t = wpool.tile([128, KT, IC], BF16, tag="wq")
                wk_t = wpool.tile([128, KT, IC], BF16, tag="wk")
                wv_t = wpool.tile([128, KT, IC], BF16, tag="wv")
                for w_t, w_dr in ((wq_t, wq_d), (wk_t, wk_d), (wv_t, wv_d)):
                    wr = w_dr.rearrange("(t p) c -> p t c", p=128)
                    for c4 in range(0, KT, 4):
                        nc.gpsimd.dma_start(out=w_t[:, c4:c4 + 4, :],
                                            in_=wr[:, c4:c4 + 4, :])

                for g in range(NGRP):
                    g0 = g * GRP
                    # ---- RMS stats chain (ACT/DVE, off the PE path) ----
                    s_cols = []
                    s_row = srpool.tile([1, GRP], F32, tag="srow")
                    for mt in range(NPT):
                        t0 = g0 + mt * 128
                        xn_t = xnpool.tile([128, D], BF16, tag="xn")
                        nc.scalar.dma_start(out=xn_t[:],
                                            in_=xn_d[t0:t0 + 128, :])
                        sq = sqpool.tile([128, D], BF16, tag="sq")
                        ss = spool.tile([128, 1], F32, tag="ss")
                        nc.scalar.activation(sq[:], xn_t[:], SQUARE,
                                             accum_out=ss[:])
                        rms = spool.tile([128, 1], F32, tag="rms")
                        nc.scalar.activation(rms[:], ss[:], SQRT,
                                             scale=1.0 / D)
                        nc.vector.tensor_scalar_max(rms[:], rms[:], EPS)
                        s_col = scolpool.tile([128, 1], F32, tag="scol")
                        nc.vector.reciprocal(s_col[:], rms[:])
                        s_cols.append(s_col)
                    # ---- xT tiles for this group ----
                    xT_g = xtpool.tile([128, KT, GRP], BF16, tag="xt")
                    for kt in range(KT):
                        eng = nc.sync if kt % 2 == 0 else nc.gpsimd
                        eng.dma_start(
                            out=xT_g[:, kt, :],
                            in_=xT_d[kt * 128:(kt + 1) * 128, g0:g0 + GRP])
                    # rope tables (raw); scaled in place once stats land
                    cos_g = cspool.tile([DH, GRP], F32, tag="cosg")
                    sin_g = cspool.tile([DH, GRP], F32, tag="sing")
                    nc.scalar.dma_start(out=cos_g[:],
                                        in_=cos_d[:, g0:g0 + GRP])
                    nc.scalar.dma_start(out=sin_g[:],
                                        in_=sin_d[:, g0:g0 + GRP])

                    emitted_sb = False
                    for w_t, dst in ((wq_t, qr), (wk_t, kr)):
                        for m in range(HPC):
                            ps = qkpool.tile([128, GRP], F32, tag="qk")
                            for kt in range(KT):
                                nc.tensor.matmul(
                                    ps[:],
                                    w_t[:, kt, m * 128:(m + 1) * 128],
                                    xT_g[:, kt, :],
                                    start=(kt == 0), stop=(kt == KT - 1))
                            if not emitted_sb:
                                # s broadcast: emitted after the first MM
                                # chain so the PE never waits on the stats
                                for mt in range(NPT):
                                    stp = stppool.tile([1, 128], F32,
                                                       tag="stp")
                                    nc.tensor.transpose(stp[:],
                                                        s_cols[mt][:],
                                                        ident[:])
                                    nc.vector.tensor_copy(
                                        s_row[:, mt * 128:(mt + 1) * 128]
                                        .bitcast(F32R),
                                        stp[:].bitcast(F32R))
                                sb_ps = sbpool.tile([128, GRP], F32,
                                                    tag="sb")
                                nc.tensor.matmul(
                                    sb_ps[:], ones_row[:].bitcast(F32R),
                                    s_row[:].bitcast(F32R),
                                    start=True, stop=True,
                                    tile_position=(0, 0))
                                nc.vector.tensor_mul(cos_g[:], cos_g[:],
                                                     sb_ps[:])
                                nc.vector.tensor_mul(sin_g[:], sin_g[:],
                                                     sb_ps[:])
                                emitted_sb = True
                            # rope: q' = q*cos(s-scaled) + rot(q)*sin
                            bb = bbpool.tile([128, GRP], F32, tag="bb")
                            nc.scalar.copy(bb[:], ps[:])
                            t1 = t1pool.tile([128, GRP], F32, tag="t1")
                            nc.vector.tensor_mul(t1[:], ps[:], cos_g[:])
                            rot = rotpool.tile([128, GRP], F32, tag="rot")
                            nc.sync.dma_start(out=rot[0:64, :],
                                              in_=bb[64:128, :])
                            nc.scalar.dma_start(out=rot[64:128, :],
                                                in_=bb[0:64, :])
                            nc.gpsimd.tensor_mul(rot[:], rot[:], sin_g[:])
                            nc.vector.tensor_add(dst[:, m, g0:g0 + GRP],
                                                 t1[:], rot[:])
                    # v projection (natural layout, scaled, into v_res)
                    for mt in range(NPT):
                        jt = g0 // 128 + mt
                        ps = vppool.tile([128, IC], F32, tag="v")
                        for kt in range(KT):
                            nc.tensor.matmul(
                                ps[:],
                                xT_g[:, kt, mt * 128:(mt + 1) * 128],
                                wv_t[:, kt, :],
                                start=(kt == 0), stop=(kt == KT - 1))
                        nc.vector.tensor_scalar_mul(v_res[:, jt, :], ps[:],
                                                    s_cols[mt][:])

            # ---------------- Phases 2+3 -------------------------------
            with tc.tile_pool(name="pat", bufs=1) as atpool:
                attnT = atpool.tile([DH, HPC, N], BF16, tag="attnT")
                wo_t = atpool.tile([128, HPC, D], BF16, tag="wo")
                nc.gpsimd.dma_start(
                    out=wo_t[:],
                    in_=wo_d.rearrange("(h p) d -> p h d", p=128))

                # ---- Phase 2: attention per head ----
                with contextlib.ExitStack() as p2st:
                    ec2 = p2st.enter_context
                    epool = ec2(tc.tile_pool(name="p2e", bufs=6))
                    r2pool = ec2(tc.tile_pool(name="p2r", bufs=2))
                    obpool = ec2(tc.tile_pool(name="p2ob", bufs=2))
                    scpool = ec2(tc.tile_pool(name="p2sc", bufs=3,
                                              space="PSUM"))
                    opool = ec2(tc.tile_pool(name="p2o", bufs=2,
                                             space="PSUM"))
                    smpool = ec2(tc.tile_pool(name="p2sum", bufs=1,
                                              space="PSUM"))
                    bcpool = ec2(tc.tile_pool(name="p2bc", bufs=1,
                                              space="PSUM"))
                    def emit_norm(pending):
                        # deferred softmax normalization for a finished
                        # (head, i-block): bc = broadcast(1/sum); attnT =
                        # o_ps * bc  (fp32 ins, bf16 out)
                        ph, pgi, po_ps, prcp = pending
                        bc_ps = bcpool.tile([128, 512], F32, tag="bc")
                        nc.tensor.matmul(bc_ps[:],
                                         ones_row[:].bitcast(F32R),
                                         prcp[:].bitcast(F32R),
                                         start=True, stop=True,
                                         tile_position=(0, 0))
                        o_sb = obpool.tile([DH, 512], F32, tag="osb")
                        nc.scalar.copy(o_sb[:], po_ps[:])
                        nc.vector.tensor_mul(
                            attnT[:, ph, pgi * 512:(pgi + 1) * 512],
                            o_sb[:], bc_ps[:])

                    def emit_sum_vacc(h, s_ps, o_ps, pend, njt):
                        j, off, ncols, e = pend
                        nc.tensor.matmul(s_ps[:, off:512], ones_col[:],
                                         e[:, :ncols],
                                         start=(j == 0), stop=(j == njt - 1),
                                         tile_position=(0, 0))
                        nc.tensor.matmul(o_ps[:, off:512],
                                         v_res[:, j, h * DH:(h + 1) * DH],
                                         e[:, :ncols],
                                         start=(j == 0), stop=(j == njt - 1))

                    pending = None
                    for h in range(HPC):
                        for gi in range(4):
                            njt = 4 * gi + 4
                            o_ps = opool.tile([DH, 512], F32, tag="o")
                            s_ps = smpool.tile([1, 512], F32, tag="sum")
                            pend = None
                            for j in range(njt):
                                off = max(0, 128 * (j - 4 * gi))
                                ncols = 512 - off
                                i0 = gi * 512 + off
                                sc = scpool.tile([128, 512], F32, tag="sc")
                                nc.tensor.matmul(
                                    sc[:, :ncols],
                                    kr[:, h, j * 128:(j + 1) * 128],
                                    qr[:, h, i0:(gi + 1) * 512],
                                    start=True, stop=True)
                                e = epool.tile([128, 512], BF16, tag="e")
                                nc.scalar.activation(e[:, :ncols],
                                                     sc[:, :ncols],
                                                     EXP, scale=ATT_SCALE)
                                if j >= 4 * gi:  # diagonal: mask triangle
                                    nc.vector.tensor_mul(e[:, 0:128],
                                                         e[:, 0:128],
                                                         mask[:])
                                if j == 1 and pending is not None:
                                    emit_norm(pending)
                                    pending = None
                                if pend is not None:
                                    emit_sum_vacc(h, s_ps, o_ps, pend, njt)
                                pend = (j, off, ncols, e)
                            emit_sum_vacc(h, s_ps, o_ps, pend, njt)
                            sum_sb = r2pool.tile([1, 512], F32, tag="ssb")
                            nc.vector.tensor_copy(sum_sb[:], s_ps[:])
                            rcp = r2pool.tile([1, 512], F32, tag="rcp")
                            with nc.allow_low_precision(reason="f32r bits"):
                                nc.vector.reciprocal(rcp[:].bitcast(F32R),
                                                     sum_sb[:])
                            pending = (h, gi, o_ps, rcp)
                    emit_norm(pending)

                # ---- Phase 3: out projection ----
                with contextlib.ExitStack() as p3st:
                    b3pool = p3st.enter_context(
                        tc.tile_pool(name="p3b", bufs=6))
                    ypool = p3st.enter_context(
                        tc.tile_pool(name="p3y", bufs=8, space="PSUM"))
                    for m in range(NT):
                        yps = [ypool.tile([128, 512], F32, tag="y",
                                          name=f"y_{m}_{n}")
                               for n in range(4)]
                        for hh in range(HPC):
                            for n in range(4):
                                nc.tensor.matmul(
                                    yps[n][:],
                                    attnT[:, hh, m * 128:(m + 1) * 128],
                                    wo_t[:, hh, n * 512:(n + 1) * 512],
                                    start=(hh == 0), stop=(hh == HPC - 1))
                        for n in range(4):
                            yb = b3pool.tile([128, 512], BF16, tag="yb")
                            if n % 2 == 0:
                                nc.vector.tensor_copy(yb[:], yps[n][:])
                            else:
                                nc.scalar.copy(yb[:], yps[n][:])
                            eng = (nc.sync, nc.scalar, nc.gpsimd,
                                   nc.sync)[n]
                            eng.dma_start(
                                out=out_d[m * 128:(m + 1) * 128,
                                          n * 512:(n + 1) * 512],
                                in_=yb[:])

    nc.compile()
    return nc


def _get_nc():
    if "nc" not in _CACHE:
        _CACHE["nc"] = _build()
    return _CACHE["nc"]


def _make_in_maps(x, rotary_emb, g, Wq, Wkv, Wo):
    import ml_dtypes

    BF = ml_dtypes.bfloat16
    x = np.asarray(x, dtype=np.float32)
    rotary_emb = np.asarray(rotary_emb, dtype=np.float32)
    g = np.asarray(g, dtype=np.float32)
    Wq = np.asarray(Wq, dtype=np.float32)
    Wkv = np.asarray(Wkv, dtype=np.float32)
    Wo = np.asarray(Wo, dtype=np.float32)

    Wqg = g[:, None] * Wq           # fold RMSNorm gain into projections
    Wkvg = g[:, None] * Wkv
    Wk = Wkvg[:, :H * DH]
    Wv = Wkvg[:, H * DH:]

    cosT = np.ascontiguousarray(np.cos(rotary_emb).T).astype(np.float32)
    sinT = np.sin(rotary_emb).T.copy()
    sinT[:64, :] *= -1.0            # sign of rotate_half folded into table
    sinTs = np.ascontiguousarray(sinT).astype(np.float32)
    maskpat = (np.arange(128)[:, None] <= np.arange(128)[None, :]).astype(BF)

    in_maps = []
    for c in range(NCORES):
        b = c // 4
        hg = c % 4
        sl = slice(hg * IC, (hg + 1) * IC)
        xb = x[b]
        in_maps.append({
            "xT": np.ascontiguousarray(xb.T).astype(BF),
            "xn": np.ascontiguousarray(xb).astype(BF),
            "wq": np.ascontiguousarray(Wqg[:, sl]).astype(BF),
            "wk": np.ascontiguousarray(Wk[:, sl]).astype(BF),
            "wv": np.ascontiguousarray(Wv[:, sl]).astype(BF),
            "wo": np.ascontiguousarray(Wo[sl, :]).astype(BF),
            "cosT": cosT,
            "sinTs": sinTs,
            "mask": maskpat,
        })
    return in_maps


def _install_ntff_hook():
    """The container's antenv stub lacks axon_hooks; synthesize it so
    run_bass_kernel_spmd(trace=True) can capture NTFF profiles."""
    import sys
    import types

    if "antenv.axon_hooks" in sys.modules:
        return
    try:
        from trn_agent_boot.trn_boot import _ntff_profile_via_ctypes
        hook = _ntff_profile_via_ctypes("/opt/axon/libaxon_pjrt.so")
    except Exception:
        hook = None
    mod = types.ModuleType("antenv.axon_hooks")
    mod.get_axon_ntff_profile_hook = lambda: hook
    mod.set_axon_ntff_profile_hook = lambda h: None
    sys.modules["antenv.axon_hooks"] = mod
    import antenv
    antenv.axon_hooks = mod


def _run(in_maps, trace=False, trace_cores=None):
    from concourse.bass_utils import run_bass_kernel_spmd

    nc = _get_nc()
    kwargs = {}
    if trace:
        _install_ntff_hook()
        kwargs = dict(trace=True, trace_cores=trace_cores or [0])
    return run_bass_kernel_spmd(nc, in_maps, list(range(NCORES)), **kwargs)


def _assemble(results):
    out = np.zeros((B, N, D), dtype=np.float64)
    for c in range(NCORES):
        out[c // 4] += np.asarray(results[c]["out"]).astype(np.float64)
    return out.astype(np.float32)


def kernel(x, rotary_emb, g, Wq, Wkv, Wo):
    in_maps = _make_in_maps(x, rotary_emb, g, Wq, Wkv, Wo)
    res = _run(in_maps)
    return _assemble(res.results)


def kernel_traced(x, rotary_emb, g, Wq, Wkv, Wo):
    """Like kernel() but also returns the profiled run (exec_time_ns)."""
    in_maps = _make_in_maps(x, rotary_emb, g, Wq, Wkv, Wo)
    res = _run(in_maps, trace=True)
    return _assemble(res.results), res


# revision 16
# speedup vs baseline: 1.2126x; 1.2126x over previous
"""Trainium2 Bass kernel for a causal multi-head attention block.

Computes (per nn.Module reference):
    xn = RMSNorm(x) * g
    q, k, v = split_heads(xn @ Wq), split_heads(xn @ Wkv)
    q, k = rope(q), rope(k)
    out = causal_softmax(q k^T / sqrt(dh)) @ v
    return merge_heads(out) @ Wo

Sharding over 8 NeuronCores: core c handles batch (c // 4) and the
4-head group (c % 4).  Each core computes its head-group's attention
output and a partial out-projection y_c = attn_heads @ Wo[head_slice];
the host sums the 4 partials per batch (the tensor-parallel
all-reduce, done on the host as part of unsharding).

v3 layout: all matmul operands bf16 (fp32 PSUM accumulation); x
arrives both natural (RMS stats) and pre-transposed (no PE
transposes); q/k/v/attn stay SBUF-resident; softmax denominators are
produced pre-broadcast by an all-ones [128,128] stationary so the
normalization never touches the PE; one dma_start per logical
transfer (SWDGE issue costs ~1us of engine time each).
"""

import contextlib
import math
import os

os.environ.setdefault("JAX_PLATFORMS", "axon")

import numpy as np

# hardcoded problem shapes (nn_Attention_369367187558)
B = 2          # batch
N = 2048       # sequence length
D = 2048       # model dim
H = 16         # heads
DH = 128       # head dim
HPC = 4        # heads per core
IC = HPC * DH  # inner dim per core (512)
NCORES = 8
GRP = 512      # token group size for phase 1
NGRP = N // GRP
KT = D // 128  # 16 contraction tiles
NT = N // 128  # 16 token tiles
EPS = 1e-8
ATT_SCALE = 1.0 / math.sqrt(DH)

_CACHE = {}


def _build():
    import concourse.mybir as mybir
    import concourse.tile as tile
    from concourse import bacc
    from concourse.masks import make_identity

    F32 = mybir.dt.float32
    F32R = mybir.dt.float32r
    BF16 = mybir.dt.bfloat16
    EXP = mybir.ActivationFunctionType.Exp
    SQRT = mybir.ActivationFunctionType.Sqrt
    SQUARE = mybir.ActivationFunctionType.Square

    nc = bacc.Bacc(None, target_bir_lowering=False)

    xT_d = nc.dram_tensor("xT", [D, N], BF16, kind="ExternalInput")
    xn_d = nc.dram_tensor("xn", [N, D], BF16, kind="ExternalInput")
    wq_d = nc.dram_tensor("wq", [D, IC], BF16, kind="ExternalInput")
    wk_d = nc.dram_tensor("wk", [D, IC], BF16, kind="ExternalInput")
    wv_d = nc.dram_tensor("wv", [D, IC], BF16, kind="ExternalInput")
    wo_d = nc.dram_tensor("wo", [IC, D], BF16, kind="ExternalInput")
    cos_d = nc.dram_tensor("cosT", [DH, N], F32, kind="ExternalInput")
    sin_d = nc.dram_tensor("sinTs", [DH, N], F32, kind="ExternalInput")
    mask_d = nc.dram_tensor("mask", [128, 128], BF16, kind="ExternalInput")
    out_d = nc.dram_tensor("out", [N, D], BF16, kind="ExternalOutput")

    NPT = GRP // 128  # token tiles per group

    with tile.TileContext(nc) as tc:
        with (
            tc.tile_pool(name="const", bufs=1) as cpool,
            tc.tile_pool(name="res", bufs=1) as respool,
        ):
            ident = cpool.tile([128, 128], F32, tag="ident")
            make_identity(nc, ident[:])
            ones_mat = cpool.tile([128, 128], BF16, tag="onesm")
            nc.vector.memset(ones_mat[:], 1.0)
            ones_rf = cpool.tile([1, 128], F32, tag="onesrf")
            nc.vector.memset(ones_rf[:], 1.0)
            ones_row = cpool.tile([1, 128], F32, tag="onesr")
            nc.vector.tensor_copy(ones_row[:].bitcast(F32R),
                                  ones_rf[:].bitcast(F32R))
            mask = cpool.tile([128, 128], BF16, tag="mask")
            nc.sync.dma_start(out=mask[:], in_=mask_d[:, :])

            # SBUF-resident between phases
            qr = respool.tile([128, HPC, N], BF16, tag="qr")
            kr = respool.tile([128, HPC, N], BF16, tag="kr")
            v_res = respool.tile([128, NT, IC], BF16, tag="vres")

            # ------- Phase 1: RMS stats + QKV + rope -------
            with contextlib.ExitStack() as p1st:
                ec = p1st.enter_context
                wpool = ec(tc.tile_pool(name="p1w", bufs=1))
                xtpool = ec(tc.tile_pool(name="p1xt", bufs=2))
                xnpool = ec(tc.tile_pool(name="p1xn", bufs=2))
                sqpool = ec(tc.tile_pool(name="p1sq", bufs=2))
                spool = ec(tc.tile_pool(name="p1s", bufs=4))
                scolpool = ec(tc.tile_pool(name="p1scol", bufs=8))
                srpool = ec(tc.tile_pool(name="p1sr", bufs=2))
                cspool = ec(tc.tile_pool(name="p1cs", bufs=2))
                bbpool = ec(tc.tile_pool(name="p1bb", bufs=3))
                t1pool = ec(tc.tile_pool(name="p1t1", bufs=3))
                rotpool = ec(tc.tile_pool(name="p1rot", bufs=3))
                qkpool = ec(tc.tile_pool(name="p1qk", bufs=3, space="PSUM"))
                vppool = ec(tc.tile_pool(name="p1v", bufs=2, space="PSUM"))
                sbpool = ec(tc.tile_pool(name="p1sb", bufs=1, space="PSUM"))
                stppool = ec(tc.tile_pool(name="p1stp", bufs=2, space="PSUM"))

                wq_t = wpool.tile([128, KT, IC], BF16, tag="wq")
                wk_t = wpool.tile([128, KT, IC], BF16, tag="wk")
                wv_t = wpool.tile([128, KT, IC], BF16, tag="wv")
                for w_t, w_dr in ((wq_t, wq_d), (wk_t, wk_d), (wv_t, wv_d)):
                    wr = w_dr.rearrange("(t p) c -> p t c", p=128)
                    for c4 in range(0, KT, 4):
                        nc.gpsimd.dma_start(out=w_t[:, c4:c4 + 4, :],
                                            in_=wr[:, c4:c4 + 4, :])
                xT_r = xT_d.rearrange("(t p) n -> p t n", p=128)
                xn_r = xn_d.rearrange("(t p) d -> p t d", p=128)

                for g in range(NGRP):
                    g0 = g * GRP
                    # ---- loads: one dma_start per logical transfer ----
                    xT_g = xtpool.tile([128, KT, GRP], BF16, tag="xt")
                    nc.gpsimd.dma_start(out=xT_g[:],
                                        in_=xT_r[:, :, g0:g0 + GRP])
                    xn_g = xnpool.tile([128, NPT, D], BF16, tag="xn")
                    nc.gpsimd.dma_start(
                        out=xn_g[:], in_=xn_r[:, g * NPT:(g + 1) * NPT, :])
                    cos_g = cspool.tile([DH, GRP], F32, tag="cosg")
                    sin_g = cspool.tile([DH, GRP], F32, tag="sing")
                    nc.scalar.dma_start(out=cos_g[:],
                                        in_=cos_d[:, g0:g0 + GRP])
                    nc.scalar.dma_start(out=sin_g[:],
                                        in_=sin_d[:, g0:g0 + GRP])
                    # ---- RMS stats (ACT/DVE, off the PE path) ----
                    sss = []
                    for mt in range(NPT):
                        sq = sqpool.tile([128, D], BF16, tag="sq")
                        ss = spool.tile([128, 1], F32, tag="ss")
                        nc.scalar.activation(sq[:], xn_g[:, mt, :], SQUARE,
                                             accum_out=ss[:])
                        sss.append(ss)
                    s_cols = []
                    s_row = srpool.tile([1, GRP], F32, tag="srow")
                    for mt in range(NPT):
                        rms = spool.tile([128, 1], F32, tag="rms")
                        nc.scalar.activation(rms[:], sss[mt][:], SQRT,
                                             scale=1.0 / D)
                        nc.vector.tensor_scalar_max(rms[:], rms[:], EPS)
                        s_col = scolpool.tile([128, 1], F32, tag="scol")
                        nc.vector.reciprocal(s_col[:], rms[:])
                        s_cols.append(s_col)

                    emitted_sb = False
                    for w_t, dst in ((wq_t, qr), (wk_t, kr)):
                        for m in range(HPC):
                            ps = qkpool.tile([128, GRP], F32, tag="qk")
                            for kt in range(KT):
                                nc.tensor.matmul(
                                    ps[:],
                                    w_t[:, kt, m * 128:(m + 1) * 128],
                                    xT_g[:, kt, :],
                                    start=(kt == 0), stop=(kt == KT - 1))
                            if not emitted_sb:
                                # s broadcast: emitted after the first MM
                                # chain so the PE never waits on the stats
                                for mt in range(NPT):
                                    stp = stppool.tile([1, 128], F32,
                                                       tag="stp")
                                    nc.tensor.transpose(stp[:],
                                                        s_cols[mt][:],
                                                        ident[:])
                                    nc.vector.tensor_copy(
                                        s_row[:, mt * 128:(mt + 1) * 128]
                                        .bitcast(F32R),
                                        stp[:].bitcast(F32R))
                                sb_ps = sbpool.tile([128, GRP], F32,
                                                    tag="sb")
                                nc.tensor.matmul(
                                    sb_ps[:], ones_row[:].bitcast(F32R),
                                    s_row[:].bitcast(F32R),
                                    start=True, stop=True,
                                    tile_position=(0, 0))
                                nc.vector.tensor_mul(cos_g[:], cos_g[:],
                                                     sb_ps[:])
                                nc.vector.tensor_mul(sin_g[:], sin_g[:],
                                                     sb_ps[:])
                                emitted_sb = True
                            # rope: q' = q*cos(s-scaled) + rot(q)*sin
                            bb = bbpool.tile([128, GRP], F32, tag="bb")
                            nc.scalar.copy(bb[:], ps[:])
                            t1 = t1pool.tile([128, GRP], F32, tag="t1")
                            nc.vector.tensor_mul(t1[:], bb[:], cos_g[:])
                            rot = rotpool.tile([128, GRP], F32, tag="rot")
                            nc.sync.dma_start(out=rot[0:64, :],
                                              in_=bb[64:128, :])
                            nc.sync.dma_start(out=rot[64:128, :],
                                              in_=bb[0:64, :])
                            nc.gpsimd.tensor_mul(rot[:], rot[:], sin_g[:])
                            nc.vector.tensor_add(dst[:, m, g0:g0 + GRP],
                                                 t1[:], rot[:])
                    # v projection (natural layout, scaled, into v_res)
                    for mt in range(NPT):
                        jt = g0 // 128 + mt
                        ps = vppool.tile([128, IC], F32, tag="v")
                        for kt in range(KT):
                            nc.tensor.matmul(
                                ps[:],
                                xT_g[:, kt, mt * 128:(mt + 1) * 128],
                                wv_t[:, kt, :],
                                start=(kt == 0), stop=(kt == KT - 1))
                        nc.vector.tensor_scalar_mul(v_res[:, jt, :], ps[:],
                                                    s_cols[mt][:])

            # ---------------- Phases 2+3 -------------------------------
            with tc.tile_pool(name="pat", bufs=1) as atpool:
                attnT = atpool.tile([DH, HPC, N], BF16, tag="attnT")
                wo_t = atpool.tile([128, HPC, D], BF16, tag="wo")
                nc.gpsimd.dma_start(
                    out=wo_t[:],
                    in_=wo_d.rearrange("(h p) d -> p h d", p=128))

                # ---- Phase 2: attention per head ----
                with contextlib.ExitStack() as p2st:
                    ec2 = p2st.enter_context
                    epool = ec2(tc.tile_pool(name="p2e", bufs=8))
                    rcpool = ec2(tc.tile_pool(name="p2rc", bufs=2))
                    obpool = ec2(tc.tile_pool(name="p2ob", bufs=2))
                    scpool = ec2(tc.tile_pool(name="p2sc", bufs=4,
                                              space="PSUM"))
                    opool = ec2(tc.tile_pool(name="p2o", bufs=2,
                                             space="PSUM"))
                    smpool = ec2(tc.tile_pool(name="p2sum", bufs=2,
                                              space="PSUM"))

                    for h in range(HPC):
                        for gi in range(4):
                            njt = 4 * gi + 4
                            o_ps = opool.tile([DH, 512], F32, tag="o")
                            sb2 = smpool.tile([128, 512], F32, tag="sum")

                            def flush(trio):
                                # sums first (shared all-ones stationary,
                                # one PSUM bank), then the attn@v trio
                                for j, off, ncols, e in trio:
                                    nc.tensor.matmul(
                                        sb2[:, off:512], ones_mat[:],
                                        e[:, :ncols],
                                        start=(j == 0), stop=(j == njt - 1))
                                for j, off, ncols, e in trio:
                                    nc.tensor.matmul(
                                        o_ps[:, off:512],
                                        v_res[:, j, h * DH:(h + 1) * DH],
                                        e[:, :ncols],
                                        start=(j == 0), stop=(j == njt - 1))

                            prev = None
                            for j0 in range(0, njt, 3):
                                trio = []
                                for j in range(j0, min(j0 + 3, njt)):
                                    off = max(0, 128 * (j - 4 * gi))
                                    ncols = 512 - off
                                    i0 = gi * 512 + off
                                    sc = scpool.tile([128, 512], F32,
                                                     tag="sc")
                                    nc.tensor.matmul(
                                        sc[:, :ncols],
                                        kr[:, h, j * 128:(j + 1) * 128],
                                        qr[:, h, i0:(gi + 1) * 512],
                                        start=True, stop=True)
                                    e = epool.tile([128, 512], BF16,
                                                   tag="e")
                                    nc.scalar.activation(e[:, :ncols],
                                                         sc[:, :ncols],
                                                         EXP,
                                                         scale=ATT_SCALE)
                                    if j >= 4 * gi:  # diagonal triangle
                                        nc.vector.tensor_mul(e[:, 0:128],
                                                             e[:, 0:128],
                                                             mask[:])
                                    trio.append((j, off, ncols, e))
                                if prev is not None:
                                    flush(prev)
                                prev = trio
                            flush(prev)
                            # deferred normalization: no PE involvement
                            rcp2 = rcpool.tile([128, 512], F32, tag="rcp")
                            nc.vector.reciprocal(rcp2[:], sb2[:])
                            o_sb = obpool.tile([DH, 512], F32, tag="osb")
                            nc.scalar.copy(o_sb[:], o_ps[:])
                            nc.gpsimd.tensor_mul(
                                attnT[:, h, gi * 512:(gi + 1) * 512],
                                o_sb[:], rcp2[:])

                # ---- Phase 3: out projection ----
                with contextlib.ExitStack() as p3st:
                    b3pool = p3st.enter_context(
                        tc.tile_pool(name="p3b", bufs=3))
                    ypool = p3st.enter_context(
                        tc.tile_pool(name="p3y", bufs=8, space="PSUM"))
                    for m in range(NT):
                        yps = [ypool.tile([128, 512], F32, tag="y",
                                          name=f"y_{m}_{n}")
                               for n in range(4)]
                        for hh in range(HPC):
                            for n in range(4):
                                nc.tensor.matmul(
                                    yps[n][:],
                                    attnT[:, hh, m * 128:(m + 1) * 128],
                                    wo_t[:, hh, n * 512:(n + 1) * 512],
                                    start=(hh == 0), stop=(hh == HPC - 1))
                        yb = b3pool.tile([128, 4, 512], BF16, tag="yb")
                        for n in range(4):
                            if n % 2 == 0:
                                nc.vector.tensor_copy(yb[:, n, :],
                                                      yps[n][:])
                            else:
                                nc.scalar.copy(yb[:, n, :], yps[n][:])
                        nc.sync.dma_start(
                            out=out_d[m * 128:(m + 1) * 128, :],
                            in_=yb[:])

    nc.compile()
    return nc


def _get_nc():
    if "nc" not in _CACHE:
        _CACHE["nc"] = _build()
    return _CACHE["nc"]


def _make_in_maps(x, rotary_emb, g, Wq, Wkv, Wo):
    import ml_dtypes

    BF = ml_dtypes.bfloat16
    x = np.asarray(x, dtype=np.float32)
    rotary_emb = np.asarray(rotary_emb, dtype=np.float32)
    g = np.asarray(g, dtype=np.float32)
    Wq = np.asarray(Wq, dtype=np.float32)
    Wkv = np.asarray(Wkv, dtype=np.float32)
    Wo = np.asarray(Wo, dtype=np.float32)

    Wqg = g[:, None] * Wq           # fold RMSNorm gain into projections
    Wkvg = g[:, None] * Wkv
    Wk = Wkvg[:, :H * DH]
    Wv = Wkvg[:, H * DH:]

    cosT = np.ascontiguousarray(np.cos(rotary_emb).T).astype(np.float32)
    sinT = np.sin(rotary_emb).T.copy()
    sinT[:64, :] *= -1.0            # sign of rotate_half folded into table
    sinTs = np.ascontiguousarray(sinT).astype(np.float32)
    maskpat = (np.arange(128)[:, None] <= np.arange(128)[None, :]).astype(BF)

    in_maps = []
    for c in range(NCORES):
        b = c // 4
        hg = c % 4
        sl = slice(hg * IC, (hg + 1) * IC)
        xb = x[b]
        in_maps.append({
            "xT": np.ascontiguousarray(xb.T).astype(BF),
            "xn": np.ascontiguousarray(xb).astype(BF),
            "wq": np.ascontiguousarray(Wqg[:, sl]).astype(BF),
            "wk": np.ascontiguousarray(Wk[:, sl]).astype(BF),
            "wv": np.ascontiguousarray(Wv[:, sl]).astype(BF),
            "wo": np.ascontiguousarray(Wo[sl, :]).astype(BF),
            "cosT": cosT,
            "sinTs": sinTs,
            "mask": maskpat,
        })
    return in_maps


def _install_ntff_hook():
    """The container's antenv stub lacks axon_hooks; synthesize it so
    run_bass_kernel_spmd(trace=True) can capture NTFF profiles."""
    import sys
    import types

    if "antenv.axon_hooks" in sys.modules:
        return
    try:
        from trn_agent_boot.trn_boot import _ntff_profile_via_ctypes
        hook = _ntff_profile_via_ctypes("/opt/axon/libaxon_pjrt.so")
    except Exception:
        hook = None
    mod = types.ModuleType("antenv.axon_hooks")
    mod.get_axon_ntff_profile_hook = lambda: hook
    mod.set_axon_ntff_profile_hook = lambda h: None
    sys.modules["antenv.axon_hooks"] = mod
    import antenv
    antenv.axon_hooks = mod


def _run(in_maps, trace=False, trace_cores=None):
    from concourse.bass_utils import run_bass_kernel_spmd

    nc = _get_nc()
    kwargs = {}
    if trace:
        _install_ntff_hook()
        kwargs = dict(trace=True, trace_cores=trace_cores or [0])
    return run_bass_kernel_spmd(nc, in_maps, list(range(NCORES)), **kwargs)


def _assemble(results):
    out = np.zeros((B, N, D), dtype=np.float64)
    for c in range(NCORES):
        out[c // 4] += np.asarray(results[c]["out"]).astype(np.float64)
    return out.astype(np.float32)


def kernel(x, rotary_emb, g, Wq, Wkv, Wo):
    in_maps = _make_in_maps(x, rotary_emb, g, Wq, Wkv, Wo)
    res = _run(in_maps)
    return _assemble(res.results)


def kernel_traced(x, rotary_emb, g, Wq, Wkv, Wo):
    """Like kernel() but also returns the profiled run (exec_time_ns)."""
    in_maps = _make_in_maps(x, rotary_emb, g, Wq, Wkv, Wo)
    res = _run(in_maps, trace=True)
    return _assemble(res.results), res


# revision 20
# speedup vs baseline: 1.3115x; 1.0815x over previous
"""Trainium2 Bass kernel for a causal multi-head attention block.

Computes (per nn.Module reference):
    xn = RMSNorm(x) * g
    q, k, v = split_heads(xn @ Wq), split_heads(xn @ Wkv)
    q, k = rope(q), rope(k)
    out = causal_softmax(q k^T / sqrt(dh)) @ v
    return merge_heads(out) @ Wo

Sharding over 8 NeuronCores: core c handles batch (c // 4) and the
4-head group (c % 4).  Each core computes its head-group's attention
output and a partial out-projection y_c = attn_heads @ Wo[head_slice];
the host sums the 4 partials per batch (the tensor-parallel
all-reduce, done on the host as part of unsharding).

v3 layout: all matmul operands bf16 (fp32 PSUM accumulation); x
arrives both natural (RMS stats) and pre-transposed (no PE
transposes); q/k/v/attn stay SBUF-resident; softmax denominators are
produced pre-broadcast by an all-ones [128,128] stationary so the
normalization never touches the PE; one dma_start per logical
transfer (SWDGE issue costs ~1us of engine time each).
"""

import contextlib
import math
import os

os.environ.setdefault("JAX_PLATFORMS", "axon")

import numpy as np

# hardcoded problem shapes (nn_Attention_369367187558)
B = 2          # batch
N = 2048       # sequence length
D = 2048       # model dim
H = 16         # heads
DH = 128       # head dim
HPC = 4        # heads per core
IC = HPC * DH  # inner dim per core (512)
NCORES = 8
GRP = 512      # token group size for phase 1
NGRP = N // GRP
KT = D // 128  # 16 contraction tiles
NT = N // 128  # 16 token tiles
EPS = 1e-8
ATT_SCALE = 1.0 / math.sqrt(DH)

_CACHE = {}


def _build():
    import concourse.mybir as mybir
    import concourse.tile as tile
    from concourse import bacc
    from concourse.masks import make_identity

    F32 = mybir.dt.float32
    F32R = mybir.dt.float32r
    BF16 = mybir.dt.bfloat16
    EXP = mybir.ActivationFunctionType.Exp
    SQRT = mybir.ActivationFunctionType.Sqrt
    SQUARE = mybir.ActivationFunctionType.Square

    nc = bacc.Bacc(None, target_bir_lowering=False)

    xT_d = nc.dram_tensor("xT", [D, N], BF16, kind="ExternalInput")
    xn_d = nc.dram_tensor("xn", [N, D], BF16, kind="ExternalInput")
    wq_d = nc.dram_tensor("wq", [D, IC], BF16, kind="ExternalInput")
    wk_d = nc.dram_tensor("wk", [D, IC], BF16, kind="ExternalInput")
    wv_d = nc.dram_tensor("wv", [D, IC], BF16, kind="ExternalInput")
    wo_d = nc.dram_tensor("wo", [IC, D], BF16, kind="ExternalInput")
    cos_d = nc.dram_tensor("cosT", [DH, N], F32, kind="ExternalInput")
    sin_d = nc.dram_tensor("sinTs", [DH, N], F32, kind="ExternalInput")
    mask_d = nc.dram_tensor("mask", [128, 128], BF16, kind="ExternalInput")
    out_d = nc.dram_tensor("out", [N, D], BF16, kind="ExternalOutput")

    NPT = GRP // 128  # token tiles per group

    with tile.TileContext(nc) as tc:
        with (
            tc.tile_pool(name="const", bufs=1) as cpool,
            tc.tile_pool(name="res", bufs=1) as respool,
        ):
            ident = cpool.tile([128, 128], F32, tag="ident")
            make_identity(nc, ident[:])
            ones_mat = cpool.tile([128, 128], BF16, tag="onesm")
            nc.vector.memset(ones_mat[:], 1.0)
            ones_rf = cpool.tile([1, 128], F32, tag="onesrf")
            nc.vector.memset(ones_rf[:], 1.0)
            ones_row = cpool.tile([1, 128], F32, tag="onesr")
            nc.vector.tensor_copy(ones_row[:].bitcast(F32R),
                                  ones_rf[:].bitcast(F32R))
            mask = cpool.tile([128, 128], BF16, tag="mask")
            nc.sync.dma_start(out=mask[:], in_=mask_d[:, :])

            # SBUF-resident between phases
            qr = respool.tile([128, HPC, N], BF16, tag="qr")
            kr = respool.tile([128, HPC, N], BF16, tag="kr")
            v_res = respool.tile([128, NT, IC], BF16, tag="vres")

            # ------- Phase 1: RMS stats + QKV + rope -------
            with contextlib.ExitStack() as p1st:
                ec = p1st.enter_context
                wpool = ec(tc.tile_pool(name="p1w", bufs=1))
                xtpool = ec(tc.tile_pool(name="p1xt", bufs=2))
                xnpool = ec(tc.tile_pool(name="p1xn", bufs=2))
                sqpool = ec(tc.tile_pool(name="p1sq", bufs=2))
                spool = ec(tc.tile_pool(name="p1s", bufs=4))
                scolpool = ec(tc.tile_pool(name="p1scol", bufs=8))
                srpool = ec(tc.tile_pool(name="p1sr", bufs=2))
                cspool = ec(tc.tile_pool(name="p1cs", bufs=2))
                bbpool = ec(tc.tile_pool(name="p1bb", bufs=3))
                t1pool = ec(tc.tile_pool(name="p1t1", bufs=3))
                rotpool = ec(tc.tile_pool(name="p1rot", bufs=3))
                qkpool = ec(tc.tile_pool(name="p1qk", bufs=3, space="PSUM"))
                vppool = ec(tc.tile_pool(name="p1v", bufs=2, space="PSUM"))
                sbpool = ec(tc.tile_pool(name="p1sb", bufs=1, space="PSUM"))
                stppool = ec(tc.tile_pool(name="p1stp", bufs=2, space="PSUM"))

                wq_t = wpool.tile([128, KT, IC], BF16, tag="wq")
                wk_t = wpool.tile([128, KT, IC], BF16, tag="wk")
                wv_t = wpool.tile([128, KT, IC], BF16, tag="wv")
                for w_t, w_dr in ((wq_t, wq_d), (wk_t, wk_d), (wv_t, wv_d)):
                    wr = w_dr.rearrange("(t p) c -> p t c", p=128)
                    for c4 in range(0, KT, 4):
                        nc.gpsimd.dma_start(out=w_t[:, c4:c4 + 4, :],
                                            in_=wr[:, c4:c4 + 4, :])
                xT_r = xT_d.rearrange("(t p) n -> p t n", p=128)
                xn_r = xn_d.rearrange("(t p) d -> p t d", p=128)

                # prefetch one group ahead on the sync ring (the first
                # qk matmul of each group is gated on its xT arrival)
                xT_gs = {}
                xn_gs = {}

                def load_group(gg):
                    xT_gs[gg] = xtpool.tile([128, KT, GRP], BF16, tag="xt",
                                            name=f"xT_g{gg}")
                    nc.sync.dma_start(
                        out=xT_gs[gg][:],
                        in_=xT_r[:, :, gg * GRP:(gg + 1) * GRP])
                    xn_gs[gg] = xnpool.tile([128, NPT, D], BF16, tag="xn",
                                            name=f"xn_g{gg}")
                    nc.sync.dma_start(
                        out=xn_gs[gg][:],
                        in_=xn_r[:, gg * NPT:(gg + 1) * NPT, :])

                load_group(0)

                for g in range(NGRP):
                    g0 = g * GRP
                    if g + 1 < NGRP:
                        load_group(g + 1)
                    xT_g = xT_gs.pop(g)
                    xn_g = xn_gs.pop(g)
                    cos_g = cspool.tile([DH, GRP], F32, tag="cosg")
                    sin_g = cspool.tile([DH, GRP], F32, tag="sing")
                    nc.scalar.dma_start(out=cos_g[:],
                                        in_=cos_d[:, g0:g0 + GRP])
                    nc.scalar.dma_start(out=sin_g[:],
                                        in_=sin_d[:, g0:g0 + GRP])
                    # ---- RMS stats (ACT/DVE, off the PE path) ----
                    sss = []
                    for mt in range(NPT):
                        sq = sqpool.tile([128, D], BF16, tag="sq")
                        ss = spool.tile([128, 1], F32, tag="ss")
                        nc.scalar.activation(sq[:], xn_g[:, mt, :], SQUARE,
                                             accum_out=ss[:])
                        sss.append(ss)
                    s_cols = []
                    s_row = srpool.tile([1, GRP], F32, tag="srow")
                    for mt in range(NPT):
                        rms = spool.tile([128, 1], F32, tag="rms")
                        nc.scalar.activation(rms[:], sss[mt][:], SQRT,
                                             scale=1.0 / D)
                        nc.vector.tensor_scalar_max(rms[:], rms[:], EPS)
                        s_col = scolpool.tile([128, 1], F32, tag="scol")
                        nc.vector.reciprocal(s_col[:], rms[:])
                        s_cols.append(s_col)

                    emitted_sb = False
                    for w_t, dst in ((wq_t, qr), (wk_t, kr)):
                        for m in range(HPC):
                            ps = qkpool.tile([128, GRP], F32, tag="qk")
                            for kt in range(KT):
                                nc.tensor.matmul(
                                    ps[:],
                                    w_t[:, kt, m * 128:(m + 1) * 128],
                                    xT_g[:, kt, :],
                                    start=(kt == 0), stop=(kt == KT - 1))
                            if not emitted_sb:
                                # s broadcast: emitted after the first MM
                                # chain so the PE never waits on the stats
                                for mt in range(NPT):
                                    stp = stppool.tile([1, 128], F32,
                                                       tag="stp")
                                    nc.tensor.transpose(stp[:],
                                                        s_cols[mt][:],
                                                        ident[:])
                                    nc.vector.tensor_copy(
                                        s_row[:, mt * 128:(mt + 1) * 128]
                                        .bitcast(F32R),
                                        stp[:].bitcast(F32R))
                                sb_ps = sbpool.tile([128, GRP], F32,
                                                    tag="sb")
                                nc.tensor.matmul(
                                    sb_ps[:], ones_row[:].bitcast(F32R),
                                    s_row[:].bitcast(F32R),
                                    start=True, stop=True,
                                    tile_position=(0, 0))
                                nc.vector.tensor_mul(cos_g[:], cos_g[:],
                                                     sb_ps[:])
                                nc.vector.tensor_mul(sin_g[:], sin_g[:],
                                                     sb_ps[:])
                                emitted_sb = True
                            # rope: q' = q*cos(s-scaled) + rot(q)*sin
                            bb = bbpool.tile([128, GRP], F32, tag="bb")
                            nc.scalar.copy(bb[:], ps[:])
                            t1 = t1pool.tile([128, GRP], F32, tag="t1")
                            nc.vector.tensor_mul(t1[:], bb[:], cos_g[:])
                            rot = rotpool.tile([128, GRP], F32, tag="rot")
                            nc.sync.dma_start(out=rot[0:64, :],
                                              in_=bb[64:128, :])
                            nc.sync.dma_start(out=rot[64:128, :],
                                              in_=bb[0:64, :])
                            nc.gpsimd.tensor_mul(rot[:], rot[:], sin_g[:])
                            nc.vector.tensor_add(dst[:, m, g0:g0 + GRP],
                                                 t1[:], rot[:])
                    # v projection (natural layout, scaled, into v_res)
                    for mt in range(NPT):
                        jt = g0 // 128 + mt
                        ps = vppool.tile([128, IC], F32, tag="v")
                        for kt in range(KT):
                            nc.tensor.matmul(
                                ps[:],
                                xT_g[:, kt, mt * 128:(mt + 1) * 128],
                                wv_t[:, kt, :],
                                start=(kt == 0), stop=(kt == KT - 1))
                        nc.vector.tensor_scalar_mul(v_res[:, jt, :], ps[:],
                                                    s_cols[mt][:])

            # ---------------- Phases 2+3 -------------------------------
            with tc.tile_pool(name="pat", bufs=1) as atpool:
                attnT = atpool.tile([DH, HPC, N], BF16, tag="attnT")
                wo_t = atpool.tile([128, HPC, D], BF16, tag="wo")
                nc.gpsimd.dma_start(
                    out=wo_t[:],
                    in_=wo_d.rearrange("(h p) d -> p h d", p=128))

                # ---- Phase 2: attention per head ----
                with contextlib.ExitStack() as p2st:
                    ec2 = p2st.enter_context
                    epool = ec2(tc.tile_pool(name="p2e", bufs=12))
                    rcpool = ec2(tc.tile_pool(name="p2rc", bufs=2))
                    scpool = ec2(tc.tile_pool(name="p2sc", bufs=5,
                                              space="PSUM"))
                    opool = ec2(tc.tile_pool(name="p2o", bufs=2,
                                             space="PSUM"))
                    smpool = ec2(tc.tile_pool(name="p2sum", bufs=1,
                                              space="PSUM"))

                    for h in range(HPC):
                        for gi in range(4):
                            njt = 4 * gi + 4
                            o_ps = opool.tile([DH, 512], F32, tag="o")
                            sb2 = smpool.tile([128, 512], F32, tag="sum")

                            def flush(trio):
                                # sums first (shared all-ones stationary,
                                # one PSUM bank), then the attn@v trio
                                for j, off, ncols, e in trio:
                                    nc.tensor.matmul(
                                        sb2[:, off:512], ones_mat[:],
                                        e[:, :ncols],
                                        start=(j == 0), stop=(j == njt - 1))
                                for j, off, ncols, e in trio:
                                    nc.tensor.matmul(
                                        o_ps[:, off:512],
                                        v_res[:, j, h * DH:(h + 1) * DH],
                                        e[:, :ncols],
                                        start=(j == 0), stop=(j == njt - 1))

                            pend_q = []
                            for j0 in range(0, njt, 3):
                                trio = []
                                for j in range(j0, min(j0 + 3, njt)):
                                    off = max(0, 128 * (j - 4 * gi))
                                    ncols = 512 - off
                                    i0 = gi * 512 + off
                                    sc = scpool.tile([128, 512], F32,
                                                     tag="sc")
                                    nc.tensor.matmul(
                                        sc[:, :ncols],
                                        kr[:, h, j * 128:(j + 1) * 128],
                                        qr[:, h, i0:(gi + 1) * 512],
                                        start=True, stop=True)
                                    e = epool.tile([128, 512], BF16,
                                                   tag="e")
                                    nc.scalar.activation(e[:, :ncols],
                                                         sc[:, :ncols],
                                                         EXP,
                                                         scale=ATT_SCALE)
                                    if j >= 4 * gi:  # diagonal triangle
                                        nc.vector.tensor_mul(e[:, 0:128],
                                                             e[:, 0:128],
                                                             mask[:])
                                    trio.append((j, off, ncols, e))
                                pend_q.append(trio)
                                if len(pend_q) > 2:
                                    flush(pend_q.pop(0))
                            for trio in pend_q:
                                flush(trio)
                            # deferred normalization: no PE involvement
                            rcp2 = rcpool.tile([128, 512], F32, tag="rcp")
                            nc.vector.reciprocal(rcp2[:], sb2[:])
                            nc.vector.tensor_mul(
                                attnT[:, h, gi * 512:(gi + 1) * 512],
                                o_ps[:], rcp2[:])

                # ---- Phase 3: out projection ----
                with contextlib.ExitStack() as p3st:
                    b3pool = p3st.enter_context(
                        tc.tile_pool(name="p3b", bufs=3))
                    ypool = p3st.enter_context(
                        tc.tile_pool(name="p3y", bufs=8, space="PSUM"))
                    for m in range(NT):
                        yps = [ypool.tile([128, 512], F32, tag="y",
                                          name=f"y_{m}_{n}")
                               for n in range(4)]
                        for hh in range(HPC):
                            for n in range(4):
                                nc.tensor.matmul(
                                    yps[n][:],
                                    attnT[:, hh, m * 128:(m + 1) * 128],
                                    wo_t[:, hh, n * 512:(n + 1) * 512],
                                    start=(hh == 0), stop=(hh == HPC - 1))
                        yb = b3pool.tile([128, 4, 512], BF16, tag="yb")
                        for n in range(4):
                            if n % 2 == 0:
                                nc.vector.tensor_copy(yb[:, n, :],
                                                      yps[n][:])
                            else:
                                nc.scalar.copy(yb[:, n, :], yps[n][:])
                        nc.sync.dma_start(
                            out=out_d[m * 128:(m + 1) * 128, :],
                            in_=yb[:])

    nc.compile()
    return nc


def _get_nc():
    if "nc" not in _CACHE:
        _CACHE["nc"] = _build()
    return _CACHE["nc"]


def _make_in_maps(x, rotary_emb, g, Wq, Wkv, Wo):
    import ml_dtypes

    BF = ml_dtypes.bfloat16
    x = np.asarray(x, dtype=np.float32)
    rotary_emb = np.asarray(rotary_emb, dtype=np.float32)
    g = np.asarray(g, dtype=np.float32)
    Wq = np.asarray(Wq, dtype=np.float32)
    Wkv = np.asarray(Wkv, dtype=np.float32)
    Wo = np.asarray(Wo, dtype=np.float32)

    Wqg = g[:, None] * Wq           # fold RMSNorm gain into projections
    Wkvg = g[:, None] * Wkv
    Wk = Wkvg[:, :H * DH]
    Wv = Wkvg[:, H * DH:]

    cosT = np.ascontiguousarray(np.cos(rotary_emb).T).astype(np.float32)
    sinT = np.sin(rotary_emb).T.copy()
    sinT[:64, :] *= -1.0            # sign of rotate_half folded into table
    sinTs = np.ascontiguousarray(sinT).astype(np.float32)
    maskpat = (np.arange(128)[:, None] <= np.arange(128)[None, :]).astype(BF)

    in_maps = []
    for c in range(NCORES):
        b = c // 4
        hg = c % 4
        sl = slice(hg * IC, (hg + 1) * IC)
        xb = x[b]
        in_maps.append({
            "xT": np.ascontiguousarray(xb.T).astype(BF),
            "xn": np.ascontiguousarray(xb).astype(BF),
            "wq": np.ascontiguousarray(Wqg[:, sl]).astype(BF),
            "wk": np.ascontiguousarray(Wk[:, sl]).astype(BF),
            "wv": np.ascontiguousarray(Wv[:, sl]).astype(BF),
            "wo": np.ascontiguousarray(Wo[sl, :]).astype(BF),
            "cosT": cosT,
            "sinTs": sinTs,
            "mask": maskpat,
        })
    return in_maps


def _install_ntff_hook():
    """The container's antenv stub lacks axon_hooks; synthesize it so
    run_bass_kernel_spmd(trace=True) can capture NTFF profiles."""
    import sys
    import types

    if "antenv.axon_hooks" in sys.modules:
        return
    try:
        from trn_agent_boot.trn_boot import _ntff_profile_via_ctypes
        hook = _ntff_profile_via_ctypes("/opt/axon/libaxon_pjrt.so")
    except Exception:
        hook = None
    mod = types.ModuleType("antenv.axon_hooks")
    mod.get_axon_ntff_profile_hook = lambda: hook
    mod.set_axon_ntff_profile_hook = lambda h: None
    sys.modules["antenv.axon_hooks"] = mod
    import antenv
    antenv.axon_hooks = mod


def _run(in_maps, trace=False, trace_cores=None):
    from concourse.bass_utils import run_bass_kernel_spmd

    nc = _get_nc()
    kwargs = {}
    if trace:
        _install_ntff_hook()
        kwargs = dict(trace=True, trace_cores=trace_cores or [0])
    return run_bass_kernel_spmd(nc, in_maps, list(range(NCORES)), **kwargs)


def _assemble(results):
    out = np.zeros((B, N, D), dtype=np.float64)
    for c in range(NCORES):
        out[c // 4] += np.asarray(results[c]["out"]).astype(np.float64)
    return out.astype(np.float32)


def kernel(x, rotary_emb, g, Wq, Wkv, Wo):
    in_maps = _make_in_maps(x, rotary_emb, g, Wq, Wkv, Wo)
    res = _run(in_maps)
    return _assemble(res.results)


def kernel_traced(x, rotary_emb, g, Wq, Wkv, Wo):
    """Like kernel() but also returns the profiled run (exec_time_ns)."""
    in_maps = _make_in_maps(x, rotary_emb, g, Wq, Wkv, Wo)
    res = _run(in_maps, trace=True)
    return _assemble(res.results), res
